# revision 18
# baseline (speedup 1.0000x reference)
"""BERT-embedding kernel for Trainium2 (8 NeuronCores, data-parallel).

Computes, for input_sequence [256,512,10], doy_sequence [256,512] (int32),
W [256,10], b [256]:

    obs = input_sequence @ W.T + b          # [256,512,256]
    pos = PE_TABLE[doy_sequence]            # [256,512,256]
    out = concat([obs, pos], axis=-1)       # [256,512,512] fp32

Strategy: shard the batch dim 8 ways (32 batches / 16384 tokens per core).
The device emits fp16 and the host upcasts to fp32 (the absmax-relative
error budget is 2e-2; fp16 rounding costs ~5e-4), halving the dominant
HBM output traffic from 32MB/core to 16MB/core.

The PE table is derived data (sinusoids of doy). Each core computes it
on the fly, with sin arguments produced NEARLY range-reduced by the
matmul itself:

  - pos (= doy-1) is decomposed into 9 signed binary digits b_i in
    {-1,0,1,2} (sum b_i 2^i = pos), chosen per position so that the
    matmul output s(tok,d) = sum_i b_i cc_i(d), with cc_i(d) =
    centered_frac(2^i * div_d / 2pi) carried hi/lo-split in the rhs,
    satisfies |s| <= 1.35 and |s + 0.25| <= 1.35 turns for every dim.
    s is congruent to pos*div_d/2pi mod 1. The cos columns add
    0.25*mask (mask=0 for doy==0 so both halves hit sin(0)=0).
  - The rhs arg columns are PRE-INTERLEAVED in output order: col
    E+2d = sin-arg(d), col E+2d+1 = cos-arg(d). Dims 0..W-1 (W=80) can
    leave [-0.5,0.5] and form a contiguous wrap block; dims W..127
    never wrap (verified over all 366 positions).
  - One fp16 matmul per 128-token column produces, in PSUM,
    [obs(256) | interleaved args(256)] per token.
  - DVE runs ONE add_range_wrap (wrap by one period into [-0.5, 0.5])
    over the 2W wrap-block args into an SBUF f tile; |args| < 1.48
    guarantees a single wrap suffices. Hardware Sin is accurate on
    [-pi, pi].
  - ACT evaluates Sin(2pi*x) with contiguous APs: a per-sub-chunk Sin
    over the PSUM no-wrap block (it only depends on the matmuls, so it
    runs early and helps the PSUM tile recycle), and ONE pair-wide Sin
    over the wrapped SBUF f tile. Both write fp16 straight into the
    output tile in final column order.
  - The obs half is copied PSUM fp32 -> SBUF fp16 split between DVE
    (cols 0..OSPLIT) and ACT (cols OSPLIT..256) to balance the two
    PSUM-capable engines.
  - One HWDGE DMA per 1024-token chunk PAIR (sync ring) writes the
    finished [128,8,512] fp16 tile; tokens are laid out so each SBUF
    partition holds 8 consecutive output rows (8KB contiguous in DRAM,
    full HBM packet efficiency).
"""

import math

import numpy as np

import concourse.bacc as bacc
import concourse.mybir as mybir
import concourse.tile as tile
from concourse.bass_utils import run_bass_kernel_spmd

F32 = mybir.dt.float32
F16 = mybir.dt.float16

# Problem shapes (hardcoded per the harness contract).
B, S, NF = 256, 512, 10
E = 256
ED2 = E // 2                      # 128 sin/cos dim pairs
MAX_LEN = 366
N_CORES = 8
TOK = (B // N_CORES) * S          # tokens per core = 16384
CPC = 4                           # 128-token cols per chunk
CH = CPC * 128                    # tokens per chunk = 512
NCH = TOK // CH                   # 32
NB = 9                            # signed binary digits of pos (0..365)
KR = NF + 1 + 2 * NB + 1          # features, ones, digit hi+lo rows, mask = 30
WD = 80                           # dim pairs [0,WD) may wrap; [WD,128) never
OSPLIT = 200                      # obs cols 0..OSPLIT on DVE, rest on ACT
TWO_PI = 2.0 * math.pi

_COMPILED_NC = None
_LAST_RESULTS = None               # BassKernelResults of the most recent run


def _build():
    nc = bacc.Bacc("TRN2", target_bir_lowering=False, debug=False)
    # XL is packed [64+KR, TOK/2]: even chunks' lhsT rows live at partitions
    # 0..KR-1, odd chunks' at 64..64+KR-1 (matmul base-partition constraint),
    # so the input load engages 2*KR partitions instead of KR.
    XL = nc.dram_tensor("XL", [64 + KR, TOK // 2], F16, kind="ExternalInput")
    RH = nc.dram_tensor("RH", [KR, 2 * E], F16, kind="ExternalInput")
    out = nc.dram_tensor("out", [TOK, 2 * E], F16, kind="ExternalOutput")

    # Token t = cc2*1024 + p*8 + jj lives at partition p, sub-chunk g2=jj//4,
    # col-group j=jj%4 of chunk pair cc2: each partition holds 8 consecutive
    # output rows (8KB contiguous in DRAM) so each 1MB pair-DMA runs at full
    # HBM packet efficiency.
    out5 = out.ap().rearrange("(cc2 p jj) e -> cc2 p jj e", p=128, jj=2 * CPC)

    with tile.TileContext(nc) as tc:
        with (
            tc.tile_pool(name="const", bufs=1) as cpool,
            tc.tile_pool(name="comb", bufs=4) as combpool,
            tc.tile_pool(name="fpool", bufs=3) as fpool,
            tc.tile_pool(name="psum", bufs=2, space="PSUM") as ppool,
        ):
            rh_sb = cpool.tile([64 + KR, 2 * E], F16, tag="rh_sb")
            nc.sync.dma_start(out=rh_sb[0:KR, :], in_=RH[:, :])
            nc.sync.dma_start(out=rh_sb[64 : 64 + KR, :], in_=RH[:, :])
            xl_sb = cpool.tile([64 + KR, TOK // 2], F16, tag="xl_sb")
            # Chunked load so early matmuls start before the full load lands;
            # each 512-col piece carries 2 chunks' worth of lhsT data.
            xl_cuts = [0, 256, 768, 1792, 3840, 8192]
            for ci in range(len(xl_cuts) - 1):
                nc.scalar.dma_start(
                    out=xl_sb[:, xl_cuts[ci] : xl_cuts[ci + 1]],
                    in_=XL[:, xl_cuts[ci] : xl_cuts[ci + 1]],
                )

            for cc2 in range(NCH // 2):
                comb = combpool.tile([128, 2, CPC, 2 * E], F16, tag="comb")
                # One f tile per pair: both sub-chunks' wraps write into it so
                # a SINGLE pair-wide Sin covers the wrapped block (fewer ACT
                # instructions -> less per-op latency tax).
                f = fpool.tile([128, 2, CPC, 2 * WD], F32, tag="f")
                for g2 in range(2):
                    ps = ppool.tile([128, CPC, 2 * E], F32, tag="ps")
                    for j in range(CPC):
                        c0 = cc2 * 512 + j * 128
                        nc.tensor.matmul(
                            out=ps[:, j, :],
                            lhsT=xl_sb[64 * g2 : 64 * g2 + KR, c0 : c0 + 128],
                            rhs=rh_sb[64 * g2 : 64 * g2 + KR, :],
                            start=True,
                            stop=True,
                        )
                    cmb = comb[:, g2]
                    # PSUM readers issue tight after the matmuls at raised
                    # priority so the PSUM tile recycles early.
                    with tc.high_priority(offset=12):
                        nc.scalar.copy(
                            out=cmb[:, :, OSPLIT:E], in_=ps[:, :, OSPLIT:E]
                        )
                        nc.vector.tensor_copy(
                            out=cmb[:, :, 0:OSPLIT], in_=ps[:, :, 0:OSPLIT]
                        )
                        nc.scalar.activation(
                            out=cmb[:, :, E + 2 * WD : 2 * E],
                            in_=ps[:, :, E + 2 * WD : 2 * E],
                            func=mybir.ActivationFunctionType.Sin,
                            scale=TWO_PI,
                        )
                        nc.vector.add_range_wrap(
                            out=f[:, g2],
                            in_=ps[:, :, E : E + 2 * WD],
                            shift=0.0,
                            bound=0.5,
                            period=1.0,
                        )
                nc.scalar.activation(
                    out=comb[:, :, :, E : E + 2 * WD],
                    in_=f[:],
                    func=mybir.ActivationFunctionType.Sin,
                    scale=TWO_PI,
                )
                nc.sync.dma_start(out=out5[cc2], in_=comb[:])
    nc.compile()
    return nc


def _digit_lut():
    """Per-position signed digits (values in {-1,0,1,2}, sum b_i 2^i = pos)
    minimizing the worst arg magnitude max(|s|, |s+0.25|) over all dims.
    Achieves <= 1.35 turns, so one period-wrap brings args into
    [-0.5, 0.5]. Wrapping dims form the contiguous prefix d < WD."""
    div = np.exp(
        np.arange(0, E, 2, dtype=np.float64) * -(math.log(10000.0) / E)
    ) / TWO_PI
    c = (2.0 ** np.arange(NB, dtype=np.float64))[:, None] * div[None, :]
    cc = c - np.round(c)

    def reps(n, nd):
        if nd == 0:
            return [[]] if n == 0 else []
        out = []
        for d in (0, 2) if n % 2 == 0 else (-1, 1):
            m = (n - d) // 2
            if -(2**nd) <= m <= 2**nd:
                for r in reps(m, nd - 1):
                    out.append([d] + r)
        return out

    lut = np.zeros((MAX_LEN, NB), np.float64)
    for pos in range(MAX_LEN):
        best = None
        for bdig in reps(pos, NB):
            bv = np.array(bdig, dtype=np.float64)
            s = bv @ cc
            margin = max(s.max(), (s + 0.25).max(), -s.min(), -(s + 0.25).min())
            key = (margin, np.abs(bv).sum())
            if best is None or key < best[0]:
                best = (key, bv)
        lut[pos] = best[1]
    # safety: dims >= WD must never leave [-0.5, 0.5] on either half
    s_all = lut @ cc
    tail = np.concatenate([s_all[:, WD:], s_all[:, WD:] + 0.25], axis=0)
    assert np.abs(tail).max() <= 0.5, np.abs(tail).max()
    assert max(np.abs(s_all).max(), np.abs(s_all + 0.25).max()) < 1.49
    return lut, cc


_DIGIT_LUT, _CC = None, None


def _host_tables(W, bias):
    """rhs [KR, 512]: obs cols = W.T rows + bias; arg cols interleaved
    (sin d, cos d) pairs carrying centered fractional digit contributions
    (fp16 hi + lo) + 0.25 cos offset on the mask row."""
    global _DIGIT_LUT, _CC
    if _DIGIT_LUT is None:
        _DIGIT_LUT, _CC = _digit_lut()
    cch = _CC.astype(np.float16)                       # [NB, 128] hi
    ccl = (_CC - cch.astype(np.float64)).astype(np.float16)  # lo residual
    RHv = np.zeros((KR, 2 * E), np.float16)
    RHv[0:NF, 0:E] = W.T.astype(np.float16)
    RHv[NF, 0:E] = bias.astype(np.float16)
    r0 = NF + 1
    RHv[r0 : r0 + NB, E + 0 :: 2] = cch                # sin cols
    RHv[r0 : r0 + NB, E + 1 :: 2] = cch                # cos cols
    RHv[r0 + NB : r0 + 2 * NB, E + 0 :: 2] = ccl
    RHv[r0 + NB : r0 + 2 * NB, E + 1 :: 2] = ccl
    RHv[r0 + 2 * NB, E + 1 :: 2] = 0.25                # cos offset via mask
    return RHv


def kernel(input_sequence, doy_sequence, W, b) -> np.ndarray:
    global _COMPILED_NC, _LAST_RESULTS

    x = np.asarray(input_sequence, dtype=np.float32)
    doy = np.asarray(doy_sequence, dtype=np.int32)
    W = np.asarray(W, dtype=np.float32)
    bias = np.asarray(b, dtype=np.float32)

    if _COMPILED_NC is None:
        _COMPILED_NC = _build()
    nc = _COMPILED_NC

    RHv = _host_tables(W, bias)
    dig16 = _DIGIT_LUT.astype(np.float16)              # [366, NB]

    bpc = B // N_CORES
    in_maps = []
    for c in range(N_CORES):
        xc = x[c * bpc : (c + 1) * bpc].reshape(TOK, NF)
        dc = doy[c * bpc : (c + 1) * bpc].reshape(TOK)
        pos = np.where(dc == 0, 0, dc - 1).astype(np.int32)
        maskf = (dc != 0).astype(np.float16)
        XLv = np.empty((KR, TOK), np.float16)
        XLv[0:NF] = xc.T.astype(np.float16)
        XLv[NF] = 1.0
        digs = dig16[pos].T                            # [NB, TOK]
        r0 = NF + 1
        XLv[r0 : r0 + NB] = digs
        XLv[r0 + NB : r0 + 2 * NB] = digs
        XLv[r0 + 2 * NB] = maskf
        # Device pair cc2, sub-chunk g2, col j, partition p holds token
        # t=cc2*1024+p*8+g2*4+j; lhsT rows live at partitions
        # 64*g2..64*g2+KR-1 and cols cc2*512+j*128+p of the packed
        # [64+KR, TOK/2] layout.
        XLv = XLv.reshape(KR, NCH // 2, 128, 2, CPC)   # r, cc2, p, g2, j
        XLv = XLv.transpose(3, 0, 1, 4, 2)             # g2, r, cc2, j, p
        XLv = XLv.reshape(2 * KR, TOK // 2)
        XLp = np.zeros((64 + KR, TOK // 2), np.float16)
        XLp[0:KR] = XLv[0:KR]
        XLp[64 : 64 + KR] = XLv[KR:]
        in_maps.append({"XL": XLp, "RH": RHv})

    _LAST_RESULTS = run_bass_kernel_spmd(nc, in_maps, core_ids=list(range(N_CORES)))

    out = np.empty((B, S, 2 * E), dtype=np.float32)
    for c in range(N_CORES):
        out[c * bpc : (c + 1) * bpc] = _LAST_RESULTS.results[c]["out"].reshape(
            bpc, S, 2 * E
        )
    return out


# revision 19
# speedup vs baseline: 1.0403x; 1.0403x over previous
"""BERT-embedding kernel for Trainium2 (8 NeuronCores, data-parallel).

Computes, for input_sequence [256,512,10], doy_sequence [256,512] (int32),
W [256,10], b [256]:

    obs = input_sequence @ W.T + b          # [256,512,256]
    pos = PE_TABLE[doy_sequence]            # [256,512,256]
    out = concat([obs, pos], axis=-1)       # [256,512,512] fp32

Strategy: shard the batch dim 8 ways (32 batches / 16384 tokens per core).
The device emits fp16 and the host upcasts to fp32 (the absmax-relative
error budget is 2e-2; fp16 rounding costs ~5e-4), halving the dominant
HBM output traffic from 32MB/core to 16MB/core.

The PE table is derived data (sinusoids of doy). Each core computes it
on the fly, with sin arguments produced NEARLY range-reduced by the
matmul itself:

  - pos (= doy-1) is decomposed into 9 signed binary digits b_i in
    {-1,0,1,2} (sum b_i 2^i = pos), chosen per position so that the
    matmul output s(tok,d) = sum_i b_i cc_i(d), with cc_i(d) =
    centered_frac(2^i * div_d / 2pi) carried hi/lo-split in the rhs,
    satisfies |s| <= 1.35 and |s + 0.25| <= 1.35 turns for every dim.
    s is congruent to pos*div_d/2pi mod 1. The cos columns add
    0.25*mask (mask=0 for doy==0 so both halves hit sin(0)=0).
  - The rhs arg columns are PRE-INTERLEAVED in output order: col
    E+2d = sin-arg(d), col E+2d+1 = cos-arg(d). Dims 0..W-1 (W=80) can
    leave [-0.5,0.5] and form a contiguous wrap block; dims W..127
    never wrap (verified over all 366 positions).
  - One fp16 matmul per 128-token column produces, in PSUM,
    [obs(256) | interleaved args(256)] per token.
  - DVE runs ONE add_range_wrap (wrap by one period into [-0.5, 0.5])
    over the 2W wrap-block args into an SBUF f tile; |args| < 1.48
    guarantees a single wrap suffices. Hardware Sin is accurate on
    [-pi, pi].
  - ACT evaluates Sin(2pi*x) with contiguous APs: a per-sub-chunk Sin
    over the PSUM no-wrap block (it only depends on the matmuls, so it
    runs early and helps the PSUM tile recycle), and ONE pair-wide Sin
    over the wrapped SBUF f tile. Both write fp16 straight into the
    output tile in final column order.
  - The obs half is copied PSUM fp32 -> SBUF fp16 split between DVE
    (cols 0..OSPLIT) and ACT (cols OSPLIT..256) to balance the two
    PSUM-capable engines.
  - One HWDGE DMA per 1024-token chunk PAIR (sync ring) writes the
    finished [128,8,512] fp16 tile; tokens are laid out so each SBUF
    partition holds 8 consecutive output rows (8KB contiguous in DRAM,
    full HBM packet efficiency).
"""

import math

import numpy as np

import concourse.bacc as bacc
import concourse.mybir as mybir
import concourse.tile as tile
from concourse.bass_utils import run_bass_kernel_spmd

F32 = mybir.dt.float32
F16 = mybir.dt.float16

# Problem shapes (hardcoded per the harness contract).
B, S, NF = 256, 512, 10
E = 256
ED2 = E // 2                      # 128 sin/cos dim pairs
MAX_LEN = 366
N_CORES = 8
TOK = (B // N_CORES) * S          # tokens per core = 16384
CPC = 4                           # 128-token cols per chunk
CH = CPC * 128                    # tokens per chunk = 512
NCH = TOK // CH                   # 32
NB = 9                            # signed binary digits of pos (0..365)
KR = NF + 1 + 2 * NB + 1          # features, ones, digit hi+lo rows, mask = 30
WD = 80                           # dim pairs [0,WD) may wrap; [WD,128) never
OSPLIT = 96                      # obs cols 0..OSPLIT on DVE, rest on ACT
TWO_PI = 2.0 * math.pi

_COMPILED_NC = None
_LAST_RESULTS = None               # BassKernelResults of the most recent run


def _build():
    nc = bacc.Bacc("TRN2", target_bir_lowering=False, debug=False)
    # XL is packed [64+KR, TOK/2]: even chunks' lhsT rows live at partitions
    # 0..KR-1, odd chunks' at 64..64+KR-1 (matmul base-partition constraint),
    # so the input load engages 2*KR partitions instead of KR.
    XL = nc.dram_tensor("XL", [64 + KR, TOK // 2], F16, kind="ExternalInput")
    RH = nc.dram_tensor("RH", [KR, 2 * E], F16, kind="ExternalInput")
    out = nc.dram_tensor("out", [TOK, 2 * E], F16, kind="ExternalOutput")

    # Token t = cc2*1024 + p*8 + jj lives at partition p, sub-chunk g2=jj//4,
    # col-group j=jj%4 of chunk pair cc2: each partition holds 8 consecutive
    # output rows (8KB contiguous in DRAM) so each 1MB pair-DMA runs at full
    # HBM packet efficiency.
    out5 = out.ap().rearrange("(cc2 p jj) e -> cc2 p jj e", p=128, jj=2 * CPC)

    with tile.TileContext(nc) as tc:
        with (
            tc.tile_pool(name="const", bufs=1) as cpool,
            tc.tile_pool(name="comb", bufs=4) as combpool,
            tc.tile_pool(name="fpool", bufs=3) as fpool,
            tc.tile_pool(name="psum", bufs=2, space="PSUM") as ppool,
        ):
            rh_sb = cpool.tile([64 + KR, 2 * E], F16, tag="rh_sb")
            nc.sync.dma_start(out=rh_sb[0:KR, :], in_=RH[:, :])
            nc.sync.dma_start(out=rh_sb[64 : 64 + KR, :], in_=RH[:, :])
            xl_sb = cpool.tile([64 + KR, TOK // 2], F16, tag="xl_sb")
            # Chunked load so early matmuls start before the full load lands;
            # each 512-col piece carries 2 chunks' worth of lhsT data.
            xl_cuts = [0, 256, 768, 1792, 3840, 8192]
            for ci in range(len(xl_cuts) - 1):
                nc.scalar.dma_start(
                    out=xl_sb[:, xl_cuts[ci] : xl_cuts[ci + 1]],
                    in_=XL[:, xl_cuts[ci] : xl_cuts[ci + 1]],
                )

            for cc2 in range(NCH // 2):
                comb = combpool.tile([128, 2, CPC, 2 * E], F16, tag="comb")
                # One f tile per pair: both sub-chunks' wraps write into it so
                # a SINGLE pair-wide Sin covers the wrapped block (fewer ACT
                # instructions -> less per-op latency tax).
                f = fpool.tile([128, 2, CPC, E], F32, tag="f")
                for g2 in range(2):
                    ps = ppool.tile([128, CPC, 2 * E], F32, tag="ps")
                    for j in range(CPC):
                        c0 = cc2 * 512 + j * 128
                        nc.tensor.matmul(
                            out=ps[:, j, :],
                            lhsT=xl_sb[64 * g2 : 64 * g2 + KR, c0 : c0 + 128],
                            rhs=rh_sb[64 * g2 : 64 * g2 + KR, :],
                            start=True,
                            stop=True,
                        )
                    cmb = comb[:, g2]
                    # PSUM readers issue tight after the matmuls at raised
                    # priority so the PSUM tile recycles early.
                    with tc.high_priority(offset=12):
                        nc.scalar.copy(
                            out=cmb[:, :, OSPLIT:E], in_=ps[:, :, OSPLIT:E]
                        )
                        nc.vector.tensor_copy(
                            out=cmb[:, :, 0:OSPLIT], in_=ps[:, :, 0:OSPLIT]
                        )
                        nc.vector.add_range_wrap(
                            out=f[:, g2],
                            in_=ps[:, :, E : 2 * E],
                            shift=0.0,
                            bound=0.5,
                            period=1.0,
                        )
                nc.scalar.activation(
                    out=comb[:, :, :, E : 2 * E],
                    in_=f[:],
                    func=mybir.ActivationFunctionType.Sin,
                    scale=TWO_PI,
                )
                nc.sync.dma_start(out=out5[cc2], in_=comb[:])
    nc.compile()
    return nc


def _digit_lut():
    """Per-position signed digits (values in {-1,0,1,2}, sum b_i 2^i = pos)
    minimizing the worst arg magnitude max(|s|, |s+0.25|) over all dims.
    Achieves <= 1.35 turns, so one period-wrap brings args into
    [-0.5, 0.5]. Wrapping dims form the contiguous prefix d < WD."""
    div = np.exp(
        np.arange(0, E, 2, dtype=np.float64) * -(math.log(10000.0) / E)
    ) / TWO_PI
    c = (2.0 ** np.arange(NB, dtype=np.float64))[:, None] * div[None, :]
    cc = c - np.round(c)

    def reps(n, nd):
        if nd == 0:
            return [[]] if n == 0 else []
        out = []
        for d in (0, 2) if n % 2 == 0 else (-1, 1):
            m = (n - d) // 2
            if -(2**nd) <= m <= 2**nd:
                for r in reps(m, nd - 1):
                    out.append([d] + r)
        return out

    lut = np.zeros((MAX_LEN, NB), np.float64)
    for pos in range(MAX_LEN):
        best = None
        for bdig in reps(pos, NB):
            bv = np.array(bdig, dtype=np.float64)
            s = bv @ cc
            margin = max(s.max(), (s + 0.25).max(), -s.min(), -(s + 0.25).min())
            key = (margin, np.abs(bv).sum())
            if best is None or key < best[0]:
                best = (key, bv)
        lut[pos] = best[1]
    # safety: dims >= WD must never leave [-0.5, 0.5] on either half
    s_all = lut @ cc
    tail = np.concatenate([s_all[:, WD:], s_all[:, WD:] + 0.25], axis=0)
    assert np.abs(tail).max() <= 0.5, np.abs(tail).max()
    assert max(np.abs(s_all).max(), np.abs(s_all + 0.25).max()) < 1.49
    return lut, cc


_DIGIT_LUT, _CC = None, None


def _host_tables(W, bias):
    """rhs [KR, 512]: obs cols = W.T rows + bias; arg cols interleaved
    (sin d, cos d) pairs carrying centered fractional digit contributions
    (fp16 hi + lo) + 0.25 cos offset on the mask row."""
    global _DIGIT_LUT, _CC
    if _DIGIT_LUT is None:
        _DIGIT_LUT, _CC = _digit_lut()
    cch = _CC.astype(np.float16)                       # [NB, 128] hi
    ccl = (_CC - cch.astype(np.float64)).astype(np.float16)  # lo residual
    RHv = np.zeros((KR, 2 * E), np.float16)
    RHv[0:NF, 0:E] = W.T.astype(np.float16)
    RHv[NF, 0:E] = bias.astype(np.float16)
    r0 = NF + 1
    RHv[r0 : r0 + NB, E + 0 :: 2] = cch                # sin cols
    RHv[r0 : r0 + NB, E + 1 :: 2] = cch                # cos cols
    RHv[r0 + NB : r0 + 2 * NB, E + 0 :: 2] = ccl
    RHv[r0 + NB : r0 + 2 * NB, E + 1 :: 2] = ccl
    RHv[r0 + 2 * NB, E + 1 :: 2] = 0.25                # cos offset via mask
    return RHv


def kernel(input_sequence, doy_sequence, W, b) -> np.ndarray:
    global _COMPILED_NC, _LAST_RESULTS

    x = np.asarray(input_sequence, dtype=np.float32)
    doy = np.asarray(doy_sequence, dtype=np.int32)
    W = np.asarray(W, dtype=np.float32)
    bias = np.asarray(b, dtype=np.float32)

    if _COMPILED_NC is None:
        _COMPILED_NC = _build()
    nc = _COMPILED_NC

    RHv = _host_tables(W, bias)
    dig16 = _DIGIT_LUT.astype(np.float16)              # [366, NB]

    bpc = B // N_CORES
    in_maps = []
    for c in range(N_CORES):
        xc = x[c * bpc : (c + 1) * bpc].reshape(TOK, NF)
        dc = doy[c * bpc : (c + 1) * bpc].reshape(TOK)
        pos = np.where(dc == 0, 0, dc - 1).astype(np.int32)
        maskf = (dc != 0).astype(np.float16)
        XLv = np.empty((KR, TOK), np.float16)
        XLv[0:NF] = xc.T.astype(np.float16)
        XLv[NF] = 1.0
        digs = dig16[pos].T                            # [NB, TOK]
        r0 = NF + 1
        XLv[r0 : r0 + NB] = digs
        XLv[r0 + NB : r0 + 2 * NB] = digs
        XLv[r0 + 2 * NB] = maskf
        # Device pair cc2, sub-chunk g2, col j, partition p holds token
        # t=cc2*1024+p*8+g2*4+j; lhsT rows live at partitions
        # 64*g2..64*g2+KR-1 and cols cc2*512+j*128+p of the packed
        # [64+KR, TOK/2] layout.
        XLv = XLv.reshape(KR, NCH // 2, 128, 2, CPC)   # r, cc2, p, g2, j
        XLv = XLv.transpose(3, 0, 1, 4, 2)             # g2, r, cc2, j, p
        XLv = XLv.reshape(2 * KR, TOK // 2)
        XLp = np.zeros((64 + KR, TOK // 2), np.float16)
        XLp[0:KR] = XLv[0:KR]
        XLp[64 : 64 + KR] = XLv[KR:]
        in_maps.append({"XL": XLp, "RH": RHv})

    _LAST_RESULTS = run_bass_kernel_spmd(nc, in_maps, core_ids=list(range(N_CORES)))

    out = np.empty((B, S, 2 * E), dtype=np.float32)
    for c in range(N_CORES):
        out[c * bpc : (c + 1) * bpc] = _LAST_RESULTS.results[c]["out"].reshape(
            bpc, S, 2 * E
        )
    return out


# revision 20
# speedup vs baseline: 1.0618x; 1.0207x over previous
"""BERT-embedding kernel for Trainium2 (8 NeuronCores, data-parallel).

Computes, for input_sequence [256,512,10], doy_sequence [256,512] (int32),
W [256,10], b [256]:

    obs = input_sequence @ W.T + b          # [256,512,256]
    pos = PE_TABLE[doy_sequence]            # [256,512,256]
    out = concat([obs, pos], axis=-1)       # [256,512,512] fp32

Strategy: shard the batch dim 8 ways (32 batches / 16384 tokens per core).
The device emits fp16 and the host upcasts to fp32 (the absmax-relative
error budget is 2e-2; fp16 rounding costs ~5e-4), halving the dominant
HBM output traffic from 32MB/core to 16MB/core.

The PE table is derived data (sinusoids of doy). Each core computes it
on the fly, with sin arguments produced NEARLY range-reduced by the
matmul itself:

  - pos (= doy-1) is decomposed into 9 signed binary digits b_i in
    {-1,0,1,2} (sum b_i 2^i = pos), chosen per position so that the
    matmul output s(tok,d) = sum_i b_i cc_i(d), with cc_i(d) =
    centered_frac(2^i * div_d / 2pi) carried hi/lo-split in the rhs,
    satisfies |s| <= 1.35 and |s + 0.25| <= 1.35 turns for every dim.
    s is congruent to pos*div_d/2pi mod 1. The cos columns add
    0.25*mask (mask=0 for doy==0 so both halves hit sin(0)=0).
  - The rhs arg columns are PRE-INTERLEAVED in output order: col
    E+2d = sin-arg(d), col E+2d+1 = cos-arg(d). Dims 0..W-1 (W=80) can
    leave [-0.5,0.5] and form a contiguous wrap block; dims W..127
    never wrap (verified over all 366 positions).
  - One fp16 matmul per 128-token column produces, in PSUM,
    [obs(256) | interleaved args(256)] per token.
  - DVE runs ONE add_range_wrap (wrap by one period into [-0.5, 0.5])
    over the 2W wrap-block args into an SBUF f tile; |args| < 1.48
    guarantees a single wrap suffices. Hardware Sin is accurate on
    [-pi, pi].
  - ACT evaluates Sin(2pi*x) with contiguous APs: a per-sub-chunk Sin
    over the PSUM no-wrap block (it only depends on the matmuls, so it
    runs early and helps the PSUM tile recycle), and ONE pair-wide Sin
    over the wrapped SBUF f tile. Both write fp16 straight into the
    output tile in final column order.
  - The obs half is copied PSUM fp32 -> SBUF fp16 split between DVE
    (cols 0..OSPLIT) and ACT (cols OSPLIT..256) to balance the two
    PSUM-capable engines.
  - One HWDGE DMA per 1024-token chunk PAIR (sync ring) writes the
    finished [128,8,512] fp16 tile; tokens are laid out so each SBUF
    partition holds 8 consecutive output rows (8KB contiguous in DRAM,
    full HBM packet efficiency).
"""

import math

import numpy as np

import concourse.bacc as bacc
import concourse.mybir as mybir
import concourse.tile as tile
from concourse.bass_utils import run_bass_kernel_spmd

F32 = mybir.dt.float32
F16 = mybir.dt.float16

# Problem shapes (hardcoded per the harness contract).
B, S, NF = 256, 512, 10
E = 256
ED2 = E // 2                      # 128 sin/cos dim pairs
MAX_LEN = 366
N_CORES = 8
TOK = (B // N_CORES) * S          # tokens per core = 16384
CPC = 4                           # 128-token cols per chunk
CH = CPC * 128                    # tokens per chunk = 512
NCH = TOK // CH                   # 32
NB = 9                            # signed binary digits of pos (0..365)
KR = NF + 1 + 2 * NB + 1          # features, ones, digit hi+lo rows, mask = 30
WD = 80                           # dim pairs [0,WD) may wrap; [WD,128) never
OSPLIT = 96                      # obs cols 0..OSPLIT on DVE, rest on ACT
TWO_PI = 2.0 * math.pi

_COMPILED_NC = None
_LAST_RESULTS = None               # BassKernelResults of the most recent run


def _build():
    nc = bacc.Bacc("TRN2", target_bir_lowering=False, debug=False)
    # XL is packed [64+KR, TOK/2]: even chunks' lhsT rows live at partitions
    # 0..KR-1, odd chunks' at 64..64+KR-1 (matmul base-partition constraint),
    # so the input load engages 2*KR partitions instead of KR.
    XL = nc.dram_tensor("XL", [64 + KR, TOK // 2], F16, kind="ExternalInput")
    RH = nc.dram_tensor("RH", [KR, 2 * E], F16, kind="ExternalInput")
    out = nc.dram_tensor("out", [TOK, 2 * E], F16, kind="ExternalOutput")

    # Token t = cc2*1024 + p*8 + jj lives at partition p, sub-chunk g2=jj//4,
    # col-group j=jj%4 of chunk pair cc2: each partition holds 8 consecutive
    # output rows (8KB contiguous in DRAM) so each 1MB pair-DMA runs at full
    # HBM packet efficiency.
    out5 = out.ap().rearrange("(cc2 p jj) e -> cc2 p jj e", p=128, jj=2 * CPC)

    with tile.TileContext(nc) as tc:
        with (
            tc.tile_pool(name="const", bufs=1) as cpool,
            tc.tile_pool(name="comb", bufs=4) as combpool,
            tc.tile_pool(name="fpool", bufs=3) as fpool,
            tc.tile_pool(name="psum", bufs=2, space="PSUM") as ppool,
        ):
            rh_sb = cpool.tile([64 + KR, 2 * E], F16, tag="rh_sb")
            nc.sync.dma_start(out=rh_sb[0:KR, :], in_=RH[:, :])
            nc.sync.dma_start(out=rh_sb[64 : 64 + KR, :], in_=RH[:, :])
            xl_sb = cpool.tile([64 + KR, TOK // 2], F16, tag="xl_sb")
            # Chunked load so early matmuls start before the full load lands;
            # each 512-col piece carries 2 chunks' worth of lhsT data.
            xl_cuts = [0, 256, 768, 1792, 3840, 8192]
            for ci in range(len(xl_cuts) - 1):
                nc.scalar.dma_start(
                    out=xl_sb[:, xl_cuts[ci] : xl_cuts[ci + 1]],
                    in_=XL[:, xl_cuts[ci] : xl_cuts[ci + 1]],
                )

            for cc2 in range(NCH // 2):
                comb = combpool.tile([128, 2, CPC, 2 * E], F16, tag="comb")
                # One f tile per pair: both sub-chunks' wraps write into it so
                # a SINGLE pair-wide Sin covers the wrapped block (fewer ACT
                # instructions -> less per-op latency tax).
                f = fpool.tile([128, 2, CPC, E], F32, tag="f")
                for g2 in range(2):
                    ps = ppool.tile([128, CPC, 2 * E], F32, tag="ps")
                    for j in range(CPC):
                        c0 = cc2 * 512 + j * 128
                        nc.tensor.matmul(
                            out=ps[:, j, :],
                            lhsT=xl_sb[64 * g2 : 64 * g2 + KR, c0 : c0 + 128],
                            rhs=rh_sb[64 * g2 : 64 * g2 + KR, :],
                            start=True,
                            stop=True,
                        )
                    cmb = comb[:, g2]
                    # PSUM readers issue tight after the matmuls at raised
                    # priority so the PSUM tile recycles early.
                    with tc.high_priority(offset=12):
                        nc.scalar.copy(
                            out=cmb[:, :, OSPLIT:E], in_=ps[:, :, OSPLIT:E]
                        )
                        nc.vector.tensor_copy(
                            out=cmb[:, :, 0:OSPLIT], in_=ps[:, :, 0:OSPLIT]
                        )
                        nc.vector.add_range_wrap(
                            out=f[:, g2],
                            in_=ps[:, :, E : 2 * E],
                            shift=0.0,
                            bound=0.5,
                            period=1.0,
                        )
                if cc2 < NCH // 2 - 1:
                    nc.scalar.activation(
                        out=comb[:, :, :, E : 2 * E],
                        in_=f[:],
                        func=mybir.ActivationFunctionType.Sin,
                        scale=TWO_PI,
                    )
                    nc.sync.dma_start(out=out5[cc2], in_=comb[:])
                else:
                    # Last pair: per-sub-chunk Sin + two half DMAs so the
                    # tail drain starts as early as possible.
                    for g2 in range(2):
                        nc.scalar.activation(
                            out=comb[:, g2, :, E : 2 * E],
                            in_=f[:, g2],
                            func=mybir.ActivationFunctionType.Sin,
                            scale=TWO_PI,
                        )
                        nc.sync.dma_start(
                            out=out5[cc2][:, g2 * CPC : (g2 + 1) * CPC],
                            in_=comb[:, g2],
                        )
    nc.compile()
    return nc


def _digit_lut():
    """Per-position signed digits (values in {-1,0,1,2}, sum b_i 2^i = pos)
    minimizing the worst arg magnitude max(|s|, |s+0.25|) over all dims.
    Achieves <= 1.35 turns, so one period-wrap brings args into
    [-0.5, 0.5]. Wrapping dims form the contiguous prefix d < WD."""
    div = np.exp(
        np.arange(0, E, 2, dtype=np.float64) * -(math.log(10000.0) / E)
    ) / TWO_PI
    c = (2.0 ** np.arange(NB, dtype=np.float64))[:, None] * div[None, :]
    cc = c - np.round(c)

    def reps(n, nd):
        if nd == 0:
            return [[]] if n == 0 else []
        out = []
        for d in (0, 2) if n % 2 == 0 else (-1, 1):
            m = (n - d) // 2
            if -(2**nd) <= m <= 2**nd:
                for r in reps(m, nd - 1):
                    out.append([d] + r)
        return out

    lut = np.zeros((MAX_LEN, NB), np.float64)
    for pos in range(MAX_LEN):
        best = None
        for bdig in reps(pos, NB):
            bv = np.array(bdig, dtype=np.float64)
            s = bv @ cc
            margin = max(s.max(), (s + 0.25).max(), -s.min(), -(s + 0.25).min())
            key = (margin, np.abs(bv).sum())
            if best is None or key < best[0]:
                best = (key, bv)
        lut[pos] = best[1]
    # safety: dims >= WD must never leave [-0.5, 0.5] on either half
    s_all = lut @ cc
    tail = np.concatenate([s_all[:, WD:], s_all[:, WD:] + 0.25], axis=0)
    assert np.abs(tail).max() <= 0.5, np.abs(tail).max()
    assert max(np.abs(s_all).max(), np.abs(s_all + 0.25).max()) < 1.49
    return lut, cc


_DIGIT_LUT, _CC = None, None


def _host_tables(W, bias):
    """rhs [KR, 512]: obs cols = W.T rows + bias; arg cols interleaved
    (sin d, cos d) pairs carrying centered fractional digit contributions
    (fp16 hi + lo) + 0.25 cos offset on the mask row."""
    global _DIGIT_LUT, _CC
    if _DIGIT_LUT is None:
        _DIGIT_LUT, _CC = _digit_lut()
    cch = _CC.astype(np.float16)                       # [NB, 128] hi
    ccl = (_CC - cch.astype(np.float64)).astype(np.float16)  # lo residual
    RHv = np.zeros((KR, 2 * E), np.float16)
    RHv[0:NF, 0:E] = W.T.astype(np.float16)
    RHv[NF, 0:E] = bias.astype(np.float16)
    r0 = NF + 1
    RHv[r0 : r0 + NB, E + 0 :: 2] = cch                # sin cols
    RHv[r0 : r0 + NB, E + 1 :: 2] = cch                # cos cols
    RHv[r0 + NB : r0 + 2 * NB, E + 0 :: 2] = ccl
    RHv[r0 + NB : r0 + 2 * NB, E + 1 :: 2] = ccl
    RHv[r0 + 2 * NB, E + 1 :: 2] = 0.25                # cos offset via mask
    return RHv


def kernel(input_sequence, doy_sequence, W, b) -> np.ndarray:
    global _COMPILED_NC, _LAST_RESULTS

    x = np.asarray(input_sequence, dtype=np.float32)
    doy = np.asarray(doy_sequence, dtype=np.int32)
    W = np.asarray(W, dtype=np.float32)
    bias = np.asarray(b, dtype=np.float32)

    if _COMPILED_NC is None:
        _COMPILED_NC = _build()
    nc = _COMPILED_NC

    RHv = _host_tables(W, bias)
    dig16 = _DIGIT_LUT.astype(np.float16)              # [366, NB]

    bpc = B // N_CORES
    in_maps = []
    for c in range(N_CORES):
        xc = x[c * bpc : (c + 1) * bpc].reshape(TOK, NF)
        dc = doy[c * bpc : (c + 1) * bpc].reshape(TOK)
        pos = np.where(dc == 0, 0, dc - 1).astype(np.int32)
        maskf = (dc != 0).astype(np.float16)
        XLv = np.empty((KR, TOK), np.float16)
        XLv[0:NF] = xc.T.astype(np.float16)
        XLv[NF] = 1.0
        digs = dig16[pos].T                            # [NB, TOK]
        r0 = NF + 1
        XLv[r0 : r0 + NB] = digs
        XLv[r0 + NB : r0 + 2 * NB] = digs
        XLv[r0 + 2 * NB] = maskf
        # Device pair cc2, sub-chunk g2, col j, partition p holds token
        # t=cc2*1024+p*8+g2*4+j; lhsT rows live at partitions
        # 64*g2..64*g2+KR-1 and cols cc2*512+j*128+p of the packed
        # [64+KR, TOK/2] layout.
        XLv = XLv.reshape(KR, NCH // 2, 128, 2, CPC)   # r, cc2, p, g2, j
        XLv = XLv.transpose(3, 0, 1, 4, 2)             # g2, r, cc2, j, p
        XLv = XLv.reshape(2 * KR, TOK // 2)
        XLp = np.zeros((64 + KR, TOK // 2), np.float16)
        XLp[0:KR] = XLv[0:KR]
        XLp[64 : 64 + KR] = XLv[KR:]
        in_maps.append({"XL": XLp, "RH": RHv})

    _LAST_RESULTS = run_bass_kernel_spmd(nc, in_maps, core_ids=list(range(N_CORES)))

    out = np.empty((B, S, 2 * E), dtype=np.float32)
    for c in range(N_CORES):
        out[c * bpc : (c + 1) * bpc] = _LAST_RESULTS.results[c]["out"].reshape(
            bpc, S, 2 * E
        )
    return out


# revision 21
# speedup vs baseline: 1.1148x; 1.0499x over previous
"""BERT-embedding kernel for Trainium2 (8 NeuronCores, data-parallel).

Computes, for input_sequence [256,512,10], doy_sequence [256,512] (int32),
W [256,10], b [256]:

    obs = input_sequence @ W.T + b          # [256,512,256]
    pos = PE_TABLE[doy_sequence]            # [256,512,256]
    out = concat([obs, pos], axis=-1)       # [256,512,512] fp32

Strategy: shard the batch dim 8 ways (32 batches / 16384 tokens per core).
The device emits fp16 and the host upcasts to fp32 (the absmax-relative
error budget is 2e-2; fp16 rounding costs ~5e-4), halving the dominant
HBM output traffic from 32MB/core to 16MB/core.

The PE table is derived data (sinusoids of doy). Each core computes it
on the fly, with sin arguments produced NEARLY range-reduced by the
matmul itself:

  - pos (= doy-1) is decomposed into 9 signed binary digits b_i in
    {-1,0,1,2} (sum b_i 2^i = pos), chosen per position so that the
    matmul output s(tok,d) = sum_i b_i cc_i(d), with cc_i(d) =
    centered_frac(2^i * div_d / 2pi) carried hi/lo-split in the rhs,
    satisfies |s| <= 1.35 and |s + 0.25| <= 1.35 turns for every dim.
    s is congruent to pos*div_d/2pi mod 1. The cos columns add
    0.25*mask (mask=0 for doy==0 so both halves hit sin(0)=0).
  - The rhs arg columns are PRE-INTERLEAVED in output order: col
    E+2d = sin-arg(d), col E+2d+1 = cos-arg(d). Dims 0..W-1 (W=80) can
    leave [-0.5,0.5] and form a contiguous wrap block; dims W..127
    never wrap (verified over all 366 positions).
  - One fp16 matmul per 128-token column produces, in PSUM,
    [obs(256) | interleaved args(256)] per token.
  - DVE runs ONE add_range_wrap (wrap by one period into [-0.5, 0.5])
    over the 2W wrap-block args into an SBUF f tile; |args| < 1.48
    guarantees a single wrap suffices. Hardware Sin is accurate on
    [-pi, pi].
  - ACT evaluates Sin(2pi*x) with contiguous APs: a per-sub-chunk Sin
    over the PSUM no-wrap block (it only depends on the matmuls, so it
    runs early and helps the PSUM tile recycle), and ONE pair-wide Sin
    over the wrapped SBUF f tile. Both write fp16 straight into the
    output tile in final column order.
  - The obs half is copied PSUM fp32 -> SBUF fp16 split between DVE
    (cols 0..OSPLIT) and ACT (cols OSPLIT..256) to balance the two
    PSUM-capable engines.
  - One HWDGE DMA per 1024-token chunk PAIR (sync ring) writes the
    finished [128,8,512] fp16 tile; tokens are laid out so each SBUF
    partition holds 8 consecutive output rows (8KB contiguous in DRAM,
    full HBM packet efficiency).
"""

import math

import numpy as np

import concourse.bacc as bacc
import concourse.mybir as mybir
import concourse.tile as tile
from concourse.bass_utils import run_bass_kernel_spmd

F32 = mybir.dt.float32
F16 = mybir.dt.float16

# Problem shapes (hardcoded per the harness contract).
B, S, NF = 256, 512, 10
E = 256
ED2 = E // 2                      # 128 sin/cos dim pairs
MAX_LEN = 366
N_CORES = 8
TOK = (B // N_CORES) * S          # tokens per core = 16384
CPC = 4                           # 128-token cols per chunk
CH = CPC * 128                    # tokens per chunk = 512
NCH = TOK // CH                   # 32
NB = 9                            # signed binary digits of pos (0..365)
KR = NF + 1 + 2 * NB + 1          # features, ones, digit hi+lo rows, mask = 30
WD = 80                           # dim pairs [0,WD) may wrap; [WD,128) never
OSPLIT = 96                      # obs cols 0..OSPLIT on DVE, rest on ACT
TWO_PI = 2.0 * math.pi

_COMPILED_NC = None
_LAST_RESULTS = None               # BassKernelResults of the most recent run


def _build():
    nc = bacc.Bacc("TRN2", target_bir_lowering=False, debug=False)
    # XL is packed [64+KR, TOK/2]: even chunks' lhsT rows live at partitions
    # 0..KR-1, odd chunks' at 64..64+KR-1 (matmul base-partition constraint),
    # so the input load engages 2*KR partitions instead of KR.
    XL = nc.dram_tensor("XL", [64 + KR, TOK // 2], F16, kind="ExternalInput")
    RH = nc.dram_tensor("RH", [KR, 2 * E], F16, kind="ExternalInput")
    out = nc.dram_tensor("out", [TOK, 2 * E], F16, kind="ExternalOutput")

    # Token t = cc2*1024 + p*8 + jj lives at partition p, sub-chunk g2=jj//4,
    # col-group j=jj%4 of chunk pair cc2: each partition holds 8 consecutive
    # output rows (8KB contiguous in DRAM) so each 1MB pair-DMA runs at full
    # HBM packet efficiency.
    out5 = out.ap().rearrange("(cc2 p jj) e -> cc2 p jj e", p=128, jj=2 * CPC)

    with tile.TileContext(nc) as tc:
        with (
            tc.tile_pool(name="const", bufs=1) as cpool,
            tc.tile_pool(name="comb", bufs=6) as combpool,
            tc.tile_pool(name="fpool", bufs=4) as fpool,
            tc.tile_pool(name="psum", bufs=2, space="PSUM") as ppool,
        ):
            rh_sb = cpool.tile([64 + KR, 2 * E], F16, tag="rh_sb")
            nc.sync.dma_start(out=rh_sb[0:KR, :], in_=RH[:, :])
            nc.sync.dma_start(out=rh_sb[64 : 64 + KR, :], in_=RH[:, :])
            xl_sb = cpool.tile([64 + KR, TOK // 2], F16, tag="xl_sb")
            # Chunked load so early matmuls start before the full load lands;
            # each 512-col piece carries 2 chunks' worth of lhsT data.
            xl_cuts = [0, 128, 512, 1280, 2816, 5376, 8192]
            for ci in range(len(xl_cuts) - 1):
                nc.scalar.dma_start(
                    out=xl_sb[:, xl_cuts[ci] : xl_cuts[ci + 1]],
                    in_=XL[:, xl_cuts[ci] : xl_cuts[ci + 1]],
                )

            for cc2 in range(NCH // 2):
                comb = combpool.tile([128, 2, CPC, 2 * E], F16, tag="comb")
                # One f tile per pair: both sub-chunks' wraps write into it so
                # a SINGLE pair-wide Sin covers the wrapped block (fewer ACT
                # instructions -> less per-op latency tax).
                f = fpool.tile([128, 2, CPC, E], F32, tag="f")
                for g2 in range(2):
                    ps = ppool.tile([128, CPC, 2 * E], F32, tag="ps")
                    for j in range(CPC):
                        c0 = cc2 * 512 + j * 128
                        nc.tensor.matmul(
                            out=ps[:, j, :],
                            lhsT=xl_sb[64 * g2 : 64 * g2 + KR, c0 : c0 + 128],
                            rhs=rh_sb[64 * g2 : 64 * g2 + KR, :],
                            start=True,
                            stop=True,
                        )
                    cmb = comb[:, g2]
                    # PSUM readers issue tight after the matmuls at raised
                    # priority so the PSUM tile recycles early.
                    with tc.high_priority(offset=12):
                        nc.scalar.copy(
                            out=cmb[:, :, OSPLIT:E], in_=ps[:, :, OSPLIT:E]
                        )
                        nc.vector.tensor_copy(
                            out=cmb[:, :, 0:OSPLIT], in_=ps[:, :, 0:OSPLIT]
                        )
                        nc.vector.add_range_wrap(
                            out=f[:, g2],
                            in_=ps[:, :, E : 2 * E],
                            shift=0.0,
                            bound=0.5,
                            period=1.0,
                        )
                if 0 < cc2 < NCH // 2 - 1:
                    nc.scalar.activation(
                        out=comb[:, :, :, E : 2 * E],
                        in_=f[:],
                        func=mybir.ActivationFunctionType.Sin,
                        scale=TWO_PI,
                    )
                    nc.sync.dma_start(out=out5[cc2], in_=comb[:])
                else:
                    # First/last pair: per-sub-chunk Sin + two half DMAs so
                    # the pipeline fills and drains as early as possible.
                    for g2 in range(2):
                        nc.scalar.activation(
                            out=comb[:, g2, :, E : 2 * E],
                            in_=f[:, g2],
                            func=mybir.ActivationFunctionType.Sin,
                            scale=TWO_PI,
                        )
                        nc.sync.dma_start(
                            out=out5[cc2][:, g2 * CPC : (g2 + 1) * CPC],
                            in_=comb[:, g2],
                        )
    nc.compile()
    return nc


def _digit_lut():
    """Per-position signed digits (values in {-1,0,1,2}, sum b_i 2^i = pos)
    minimizing the worst arg magnitude max(|s|, |s+0.25|) over all dims.
    Achieves <= 1.35 turns, so one period-wrap brings args into
    [-0.5, 0.5]. Wrapping dims form the contiguous prefix d < WD."""
    div = np.exp(
        np.arange(0, E, 2, dtype=np.float64) * -(math.log(10000.0) / E)
    ) / TWO_PI
    c = (2.0 ** np.arange(NB, dtype=np.float64))[:, None] * div[None, :]
    cc = c - np.round(c)

    def reps(n, nd):
        if nd == 0:
            return [[]] if n == 0 else []
        out = []
        for d in (0, 2) if n % 2 == 0 else (-1, 1):
            m = (n - d) // 2
            if -(2**nd) <= m <= 2**nd:
                for r in reps(m, nd - 1):
                    out.append([d] + r)
        return out

    lut = np.zeros((MAX_LEN, NB), np.float64)
    for pos in range(MAX_LEN):
        best = None
        for bdig in reps(pos, NB):
            bv = np.array(bdig, dtype=np.float64)
            s = bv @ cc
            margin = max(s.max(), (s + 0.25).max(), -s.min(), -(s + 0.25).min())
            key = (margin, np.abs(bv).sum())
            if best is None or key < best[0]:
                best = (key, bv)
        lut[pos] = best[1]
    # safety: dims >= WD must never leave [-0.5, 0.5] on either half
    s_all = lut @ cc
    tail = np.concatenate([s_all[:, WD:], s_all[:, WD:] + 0.25], axis=0)
    assert np.abs(tail).max() <= 0.5, np.abs(tail).max()
    assert max(np.abs(s_all).max(), np.abs(s_all + 0.25).max()) < 1.49
    return lut, cc


_DIGIT_LUT, _CC = None, None


def _host_tables(W, bias):
    """rhs [KR, 512]: obs cols = W.T rows + bias; arg cols interleaved
    (sin d, cos d) pairs carrying centered fractional digit contributions
    (fp16 hi + lo) + 0.25 cos offset on the mask row."""
    global _DIGIT_LUT, _CC
    if _DIGIT_LUT is None:
        _DIGIT_LUT, _CC = _digit_lut()
    cch = _CC.astype(np.float16)                       # [NB, 128] hi
    ccl = (_CC - cch.astype(np.float64)).astype(np.float16)  # lo residual
    RHv = np.zeros((KR, 2 * E), np.float16)
    RHv[0:NF, 0:E] = W.T.astype(np.float16)
    RHv[NF, 0:E] = bias.astype(np.float16)
    r0 = NF + 1
    RHv[r0 : r0 + NB, E + 0 :: 2] = cch                # sin cols
    RHv[r0 : r0 + NB, E + 1 :: 2] = cch                # cos cols
    RHv[r0 + NB : r0 + 2 * NB, E + 0 :: 2] = ccl
    RHv[r0 + NB : r0 + 2 * NB, E + 1 :: 2] = ccl
    RHv[r0 + 2 * NB, E + 1 :: 2] = 0.25                # cos offset via mask
    return RHv


def kernel(input_sequence, doy_sequence, W, b) -> np.ndarray:
    global _COMPILED_NC, _LAST_RESULTS

    x = np.asarray(input_sequence, dtype=np.float32)
    doy = np.asarray(doy_sequence, dtype=np.int32)
    W = np.asarray(W, dtype=np.float32)
    bias = np.asarray(b, dtype=np.float32)

    if _COMPILED_NC is None:
        _COMPILED_NC = _build()
    nc = _COMPILED_NC

    RHv = _host_tables(W, bias)
    dig16 = _DIGIT_LUT.astype(np.float16)              # [366, NB]

    bpc = B // N_CORES
    in_maps = []
    for c in range(N_CORES):
        xc = x[c * bpc : (c + 1) * bpc].reshape(TOK, NF)
        dc = doy[c * bpc : (c + 1) * bpc].reshape(TOK)
        pos = np.where(dc == 0, 0, dc - 1).astype(np.int32)
        maskf = (dc != 0).astype(np.float16)
        XLv = np.empty((KR, TOK), np.float16)
        XLv[0:NF] = xc.T.astype(np.float16)
        XLv[NF] = 1.0
        digs = dig16[pos].T                            # [NB, TOK]
        r0 = NF + 1
        XLv[r0 : r0 + NB] = digs
        XLv[r0 + NB : r0 + 2 * NB] = digs
        XLv[r0 + 2 * NB] = maskf
        # Device pair cc2, sub-chunk g2, col j, partition p holds token
        # t=cc2*1024+p*8+g2*4+j; lhsT rows live at partitions
        # 64*g2..64*g2+KR-1 and cols cc2*512+j*128+p of the packed
        # [64+KR, TOK/2] layout.
        XLv = XLv.reshape(KR, NCH // 2, 128, 2, CPC)   # r, cc2, p, g2, j
        XLv = XLv.transpose(3, 0, 1, 4, 2)             # g2, r, cc2, j, p
        XLv = XLv.reshape(2 * KR, TOK // 2)
        XLp = np.zeros((64 + KR, TOK // 2), np.float16)
        XLp[0:KR] = XLv[0:KR]
        XLp[64 : 64 + KR] = XLv[KR:]
        in_maps.append({"XL": XLp, "RH": RHv})

    _LAST_RESULTS = run_bass_kernel_spmd(nc, in_maps, core_ids=list(range(N_CORES)))

    out = np.empty((B, S, 2 * E), dtype=np.float32)
    for c in range(N_CORES):
        out[c * bpc : (c + 1) * bpc] = _LAST_RESULTS.results[c]["out"].reshape(
            bpc, S, 2 * E
        )
    return out
